# revision 21
# baseline (speedup 1.0000x reference)
"""Trainium2 Bass kernel for a Restormer-style block (MDTA channel attention + GDFN).

Sharding: pure data parallel, one sample per NeuronCore (B=8 samples, 8 cores).
Per-core layout: channels on partitions, flattened spatial on free dim: [128, 16384].

Per core:
  GN1 stats -> fold (alpha,beta) into W_qkv -> tiled: pw matmul -> zero-padded row
  strips -> depthwise 3x3 (fused shift-MAC taps on DVE / diagonal-matmul PSUM
  accumulation on PE) -> q,k,v. l2 row norms + Gram(q,k) via PE transposes ->
  softmax -> M^T = A^T (attn_scale*Wproj)^T -> x2 = x + M@v -> GN2 -> fold into
  W_12 -> tiled: pw -> dw3x3 -> gelu-gate -> pw_out -> out = x2 + ff.
"""

import numpy as np
import ml_dtypes

import concourse.bass as bass
import concourse.bacc as bacc
import concourse.mybir as mybir
import concourse.tile as tile
from concourse.bass_utils import run_bass_kernel_spmd

F32 = mybir.dt.float32
BF16 = mybir.dt.bfloat16
AF = mybir.ActivationFunctionType
ALU = mybir.AluOpType
AX = mybir.AxisListType

B = 8
C = 128
HEADS = 8
HD = C // HEADS          # 16
HH = 128                 # image height
WW = 128                 # image width
N = HH * WW              # 16384
MID = int(C * 2.66)      # 340
M2 = 2 * MID             # 680
EPS_GN = 1e-5
EPS_L2 = 1e-12

R = 16                   # image rows per tile
TILES = HH // R
SR = R + 2               # strip rows (halo)
SW = WW + 2              # strip width (zero-pad cols)
NB = R * WW // 512       # 512-col blocks per tile (4)

# ff chunking aligned so (t1_i, t2_i) pairs share partitions
FF_SIZES = [128, 128, 84, 128, 128, 84]
FF_OFFS = [0, 128, 256, 340, 468, 596]

PH1_OWN = {"q": "dve", "k": "pe", "v": "pe"}
PH2_OWN = ["dve", "dve", "pe", "pe", "pe", "pe"]
# leading taps per ff chunk computed on PE (rest on DVE, merged via TT add)
PH2_TAPS_PE = [0, 5, 9, 9, 9, 9]

_CACHED = {}


def _mm_blocks(lo, hi, step=512):
    out = []
    a = lo
    while a < hi:
        out.append((a, min(step, hi - a)))
        a += step
    return out


def _build(iters=1):
    key = ("nc", iters)
    if key in _CACHED:
        return _CACHED[key]
    nc = bacc.Bacc("TRN2", target_bir_lowering=False, debug=False, num_devices=B)

    T = {}
    T["x"] = nc.dram_tensor("x", [C, N], F32, kind="ExternalInput").ap()
    T["wqkvT"] = nc.dram_tensor("wqkvT", [C, 3 * C], F32, kind="ExternalInput").ap()
    T["dwqkv"] = nc.dram_tensor("dwqkv", [3 * C, 9], F32, kind="ExternalInput").ap()
    T["projT_s"] = nc.dram_tensor("projT_s", [C, C], F32, kind="ExternalInput").ap()
    T["w12T"] = nc.dram_tensor("w12T", [C, M2], F32, kind="ExternalInput").ap()
    T["dw12"] = nc.dram_tensor("dw12", [M2, 9], F32, kind="ExternalInput").ap()
    T["pwoT_s"] = nc.dram_tensor("pwoT_s", [MID, C], F32, kind="ExternalInput").ap()
    for nm in ("n1w", "n1b", "n2w", "n2b", "tau"):
        T[nm] = nc.dram_tensor(nm, [C, 1], F32, kind="ExternalInput").ap()
    T["ident"] = nc.dram_tensor("ident", [C, C], BF16, kind="ExternalInput").ap()
    T["hmask"] = nc.dram_tensor("hmask", [C, C], F32, kind="ExternalInput").ap()
    T["onesm"] = nc.dram_tensor("onesm", [C, C], F32, kind="ExternalInput").ap()
    T["out"] = nc.dram_tensor("out", [C, N], F32, kind="ExternalOutput").ap()

    with tile.TileContext(nc) as tc:
        for _ in range(iters):
            _emit(tc, T)
    nc.compile()
    _CACHED[key] = nc
    return nc


class Env:
    pass


def _gn_chunk(tc, e, x, xb, j, scr_pool):
    """GN stats + bf16 cast for 4096-col chunk j."""
    nc = tc.nc
    sl = slice(4096 * j, 4096 * (j + 1))
    nc.vector.tensor_reduce(e.s1p[:, j:j + 1], x[:, sl], AX.X, ALU.add)
    scr = scr_pool.tile([C, 4096], BF16, tag="gnscr")
    nc.scalar.activation(scr[:], x[:, sl], AF.Square, accum_out=e.s2p[:, j:j + 1])
    nc.vector.tensor_copy(xb[:, sl], x[:, sl])


def _gn_combine(tc, e, nw, nb_, ps_pool):
    nc = tc.nc
    nc.vector.tensor_reduce(e.stats2[:, 0:1], e.s1p[:], AX.X, ALU.add)
    nc.vector.tensor_reduce(e.stats2[:, 1:2], e.s2p[:], AX.X, ALU.add)
    sps = ps_pool.tile([C, 2], F32, tag="small")
    nc.tensor.matmul(sps[:], e.onesf[:], e.stats2[:], start=True, stop=True)
    nc.vector.tensor_copy(e.sums[:], sps[:])
    inv = 1.0 / (C * N)
    nc.vector.tensor_scalar_mul(e.mean[:], e.sums[:, 0:1], inv)
    nc.vector.tensor_scalar_mul(e.ex2[:], e.sums[:, 1:2], inv)
    nc.vector.tensor_mul(e.msq[:], e.mean[:], e.mean[:])
    nc.vector.tensor_sub(e.var[:], e.ex2[:], e.msq[:])
    nc.vector.tensor_scalar_add(e.var[:], e.var[:], EPS_GN)
    nc.scalar.activation(e.std[:], e.var[:], AF.Sqrt)
    nc.vector.reciprocal(e.rstd[:], e.std[:])
    nc.vector.tensor_mul(e.alpha[:], nw[:], e.rstd[:])
    nc.vector.tensor_mul(e.tmp1[:], e.mean[:], e.alpha[:])
    nc.vector.tensor_sub(e.beta[:], nb_[:], e.tmp1[:])


def _gn_fold(tc, e, x, xb, nw, nb_, scr_pool, ps_pool):
    for j in range(4):
        _gn_chunk(tc, e, x, xb, j, scr_pool)
    _gn_combine(tc, e, nw, nb_, ps_pool)


def _strip_prep(nc, strip, sz, t):
    sv = strip[:sz, :].rearrange("p (r w) -> p r w", w=SW)
    nc.vector.memset(sv[:, :, 0:1], 0.0)
    nc.vector.memset(sv[:, :, SW - 1:SW], 0.0)
    if t == 0:
        nc.vector.memset(sv[:, 0:1, :], 0.0)
    if t == TILES - 1:
        nc.vector.memset(sv[:, SR - 1:SR, :], 0.0)
    return sv


def _win(sv, i, r0, nr):
    dy, dx = i // 3, i % 3
    return sv[:, dy + r0:dy + r0 + nr, dx:dx + WW]


def _emit(tc, T):
    nc = tc.nc
    e = Env()

    with tc.tile_pool(name="persist", bufs=1) as P:
        x = P.tile([C, N], F32, tag="x")
        xb = P.tile([C, N], BF16, tag="xb")

        e.identb = P.tile([C, C], BF16, tag="identb")
        e.onesf = P.tile([C, C], F32, tag="onesf")
        projb = P.tile([C, C], BF16, tag="projb")
        wqkvb = P.tile([C, 3 * C], BF16, tag="wqkvb")
        w12b = P.tile([C, M2], BF16, tag="w12b")
        pwo_b = [P.tile([ks, C], BF16, tag=f"pwo{i}", name=f"pwo{i}") for i, ks in enumerate((128, 128, 84))]
        dwq = [P.tile([C, 9], F32, tag=f"dwq{i}", name=f"dwq{i}") for i in range(3)]
        dw12t = [P.tile([FF_SIZES[i], 9], F32, tag=f"dw12_{i}", name=f"dw12_{i}") for i in range(6)]
        biasq = [P.tile([C, 1], F32, tag=f"biasq{i}", name=f"biasq{i}") for i in range(3)]
        bias12 = [P.tile([FF_SIZES[i], 1], F32, tag=f"bias12_{i}", name=f"bias12_{i}") for i in range(6)]

        n1w = P.tile([C, 1], F32, tag="n1w")
        n1b = P.tile([C, 1], F32, tag="n1b")
        n2w = P.tile([C, 1], F32, tag="n2w")
        n2b = P.tile([C, 1], F32, tag="n2b")
        tau = P.tile([C, 1], F32, tag="tau")

        for nm in ("s1p", "s2p"):
            setattr(e, nm, P.tile([C, 4], F32, tag=nm, name=nm))
        e.stats2 = P.tile([C, 2], F32, tag="stats2")
        e.sums = P.tile([C, 2], F32, tag="sums")
        for nm in ("mean", "ex2", "msq", "var", "std", "rstd", "alpha", "beta",
                   "tmp1", "rq", "rk", "sumE", "rE"):
            setattr(e, nm, P.tile([C, 1], F32, tag=nm, name=nm))
        sqp = P.tile([C, TILES], F32, tag="sqp")
        skp = P.tile([C, TILES], F32, tag="skp")
        diagk = [P.tile([C, C], BF16, tag=f"diagk{i}", name=f"diagk{i}") for i in range(9)]
        diagv = [P.tile([C, C], BF16, tag=f"diagv{i}", name=f"diagv{i}") for i in range(9)]
        gram_s = P.tile([C, C], F32, tag="gram_s")
        gA = P.tile([C, C], BF16, tag="gA")
        gB = P.tile([C, C], BF16, tag="gB")
        Ef = P.tile([C, C], F32, tag="Ef")
        hmask = P.tile([C, C], F32, tag="hmask")
        A_full = P.tile([C, C], BF16, tag="A_full")
        mT = P.tile([C, C], BF16, tag="mT")

        # ---- constant DMAs ----
        nc.sync.dma_start(e.identb[:], T["ident"])
        nc.sync.dma_start(hmask[:], T["hmask"])
        nc.sync.dma_start(e.onesf[:], T["onesm"])
        for nm, t_ in (("n1w", n1w), ("n1b", n1b), ("n2w", n2w), ("n2b", n2b),
                       ("tau", tau)):
            nc.sync.dma_start(t_[:], T[nm])
        for i in range(3):
            nc.sync.dma_start(dwq[i][:], T["dwqkv"][128 * i:128 * (i + 1), :])
        for i in range(6):
            nc.sync.dma_start(dw12t[i][:],
                              T["dw12"][FF_OFFS[i]:FF_OFFS[i] + FF_SIZES[i], :])

        with tc.tile_pool(name="vpool", bufs=1) as VP:
            v_full = VP.tile([C, N], BF16, tag="v_full")

            # ================= setup: load x, GN1, fold =================
            with tc.tile_pool(name="setup", bufs=2) as SP, \
                 tc.tile_pool(name="pss", bufs=2, space="PSUM") as PSS:
                ko = [0, 128, 256]
                for i, ks in enumerate((128, 128, 84)):
                    pwof = SP.tile([ks, C], F32, tag="pwof")
                    nc.sync.dma_start(pwof[:], T["pwoT_s"][ko[i]:ko[i] + ks, :])
                    nc.vector.tensor_copy(pwo_b[i][:], pwof[:ks, :])
                projf = SP.tile([C, C], F32, tag="projf")
                nc.sync.dma_start(projf[:], T["projT_s"])
                nc.vector.tensor_copy(projb[:], projf[:])

                for j in range(4):
                    sl = slice(4096 * j, 4096 * (j + 1))
                    nc.sync.dma_start(x[:, sl], T["x"][:, sl])
                _gn_fold(tc, e, x, xb, n1w, n1b, SP, PSS)

                wtmp = SP.tile([C, 3 * C], F32, tag="wtmp")
                nc.sync.dma_start(wtmp[:], T["wqkvT"])
                nc.vector.tensor_scalar(wqkvb[:], wtmp[:], e.alpha[:], None, ALU.mult)
                for ci in range(3):
                    bp = PSS.tile([C, 1], F32, tag="small")
                    nc.tensor.matmul(bp[:], wtmp[:, 128 * ci:128 * (ci + 1)], e.beta[:],
                                     start=True, stop=True)
                    nc.vector.tensor_copy(biasq[ci][:], bp[:])
                for i in range(9):
                    nc.vector.tensor_scalar(diagk[i][:], e.identb[:], dwq[1][:, i:i + 1],
                                            None, ALU.mult)
                    nc.vector.tensor_scalar(diagv[i][:], e.identb[:], dwq[2][:, i:i + 1],
                                            None, ALU.mult)

            # ================= phase 1: MDTA =================
            with tc.tile_pool(name="gramp", bufs=1, space="PSUM") as GRAMP:
                gram = GRAMP.tile([C, C], F32, tag="gram")
                with tc.tile_pool(name="ph1", bufs=2) as H1, \
                     tc.tile_pool(name="ph1mm", bufs=3, space="PSUM") as MMP, \
                     tc.tile_pool(name="ph1cv", bufs=2, space="PSUM") as CVP, \
                     tc.tile_pool(name="ph1tp", bufs=2, space="PSUM") as TPP:
                    for t in range(TILES):
                        mm_lo = max(t * R - 1, 0) * WW
                        mm_hi = min((t + 1) * R + 1, HH) * WW
                        c0 = t * R * WW

                        strips = []
                        for name in "qkv":
                            s = H1.tile([C, SR * SW], BF16, tag=f"s{name}", name=f"s{name}")
                            strips.append(_strip_prep(nc, s, C, t))

                        for (a0, ncols) in _mm_blocks(mm_lo, mm_hi):
                            nrows = ncols // WW
                            sr0 = a0 // WW - t * R + 1
                            for ci in range(3):
                                ps = MMP.tile([C, 512], F32, tag="mm")
                                nc.tensor.matmul(
                                    ps[:, :ncols],
                                    wqkvb[:, 128 * ci:128 * (ci + 1)],
                                    xb[:, a0:a0 + ncols], start=True, stop=True)
                                nc.scalar.activation(
                                    strips[ci][:, sr0:sr0 + nrows, 1:1 + WW],
                                    ps[:, :ncols].rearrange("p (r w) -> p r w", w=WW),
                                    AF.Identity, bias=biasq[ci][:])

                        # q: DVE fused taps
                        q_acc = H1.tile([C, R * WW], BF16, tag="q_acc")
                        qv = q_acc[:].rearrange("p (r w) -> p r w", w=WW)
                        nc.vector.tensor_scalar(qv, _win(strips[0], 0, 0, R),
                                                dwq[0][:, 0:1], None, ALU.mult)
                        for i in range(1, 9):
                            nc.vector.scalar_tensor_tensor(
                                qv, _win(strips[0], i, 0, R), dwq[0][:, i:i + 1],
                                qv, ALU.mult, ALU.add)

                        # k, v: PE diagonal-matmul taps
                        k_acc = H1.tile([C, R * WW], BF16, tag="k_acc")
                        for b in range(NB):
                            r0 = 4 * b
                            pk = CVP.tile([C, 512], F32, tag="cv")
                            for i in range(9):
                                nc.tensor.matmul(pk[:], diagk[i][:],
                                                 _win(strips[1], i, r0, 4),
                                                 start=(i == 0), stop=(i == 8))
                            nc.scalar.copy(k_acc[:, 512 * b:512 * (b + 1)], pk[:])
                            pv = CVP.tile([C, 512], F32, tag="cv")
                            for i in range(9):
                                nc.tensor.matmul(pv[:], diagv[i][:],
                                                 _win(strips[2], i, r0, 4),
                                                 start=(i == 0), stop=(i == 8))
                            nc.scalar.copy(v_full[:, c0 + 512 * b:c0 + 512 * (b + 1)],
                                           pv[:])

                        # transposes + Gram; qT/kT double as Square scratch
                        qT = H1.tile([C, R * WW], BF16, tag="qT")
                        kT = H1.tile([C, R * WW], BF16, tag="kT")
                        nc.scalar.activation(qT[:], q_acc[:], AF.Square,
                                             accum_out=sqp[:, t:t + 1])
                        nc.scalar.activation(kT[:], k_acc[:], AF.Square,
                                             accum_out=skp[:, t:t + 1])
                        ntp = R * WW // C
                        for j4 in range(ntp // 4):
                            tq = TPP.tile([C, 512], BF16, tag="tp")
                            tk = TPP.tile([C, 512], BF16, tag="tp")
                            for jj in range(4):
                                j = 4 * j4 + jj
                                nc.tensor.transpose(tq[:, 128 * jj:128 * (jj + 1)],
                                                    q_acc[:, 128 * j:128 * (j + 1)],
                                                    e.identb[:])
                                nc.tensor.transpose(tk[:, 128 * jj:128 * (jj + 1)],
                                                    k_acc[:, 128 * j:128 * (j + 1)],
                                                    e.identb[:])
                            nc.scalar.copy(qT[:, 512 * j4:512 * (j4 + 1)], tq[:])
                            nc.scalar.copy(kT[:, 512 * j4:512 * (j4 + 1)], tk[:])
                        for j in range(ntp):
                            nc.tensor.matmul(
                                gram[:], qT[:, 128 * j:128 * (j + 1)],
                                kT[:, 128 * j:128 * (j + 1)],
                                start=(t == 0 and j == 0),
                                stop=(t == TILES - 1 and j == ntp - 1))

                # l2 norms -> rq, rk (rk includes temperature)
                for parts, dst in ((sqp, e.rq), (skp, e.rk)):
                    nc.vector.tensor_reduce(dst[:], parts[:], AX.X, ALU.add)
                    nc.scalar.activation(dst[:], dst[:], AF.Sqrt)
                    nc.vector.tensor_scalar_max(dst[:], dst[:], EPS_L2)
                    nc.vector.reciprocal(dst[:], dst[:])
                nc.vector.tensor_mul(e.rk[:], e.rk[:], tau[:])
                nc.vector.tensor_copy(gram_s[:], gram[:])

            # ============ attention finalize + x2 + GN2 ============
            with tc.tile_pool(name="attp", bufs=2, space="PSUM") as ATP, \
                 tc.tile_pool(name="midp", bufs=2) as MP:
                nc.vector.tensor_scalar(gA[:], gram_s[:], e.rq[:], None, ALU.mult)
                tp1 = ATP.tile([C, C], BF16, tag="tpa")
                nc.tensor.transpose(tp1[:], gA[:], e.identb[:])
                nc.vector.tensor_scalar(gB[:], tp1[:], e.rk[:], None, ALU.mult)
                tp2 = ATP.tile([C, C], BF16, tag="tpa")
                nc.tensor.transpose(tp2[:], gB[:], e.identb[:])
                nc.scalar.activation(Ef[:], tp2[:], AF.Exp)
                nc.vector.tensor_tensor(Ef[:], Ef[:], hmask[:], ALU.mult)
                nc.vector.tensor_reduce(e.sumE[:], Ef[:], AX.X, ALU.add)
                nc.vector.reciprocal(e.rE[:], e.sumE[:])
                nc.vector.tensor_scalar(A_full[:], Ef[:], e.rE[:], None, ALU.mult)
                mtp = ATP.tile([C, C], F32, tag="tpa")
                nc.tensor.matmul(mtp[:], A_full[:], projb[:], start=True, stop=True)
                nc.scalar.copy(mT[:], mtp[:])

                with tc.tile_pool(name="pss2", bufs=2, space="PSUM") as PSS2:
                    for a0 in range(0, N, 512):
                        ap_ = ATP.tile([C, 512], F32, tag="apv", bufs=4)
                        nc.tensor.matmul(ap_[:], mT[:], v_full[:, a0:a0 + 512],
                                         start=True, stop=True)
                        nc.vector.tensor_tensor(x[:, a0:a0 + 512], x[:, a0:a0 + 512],
                                                ap_[:], ALU.add)
                        if a0 % 4096 == 4096 - 512:
                            _gn_chunk(tc, e, x, xb, a0 // 4096, MP)
                    _gn_combine(tc, e, n2w, n2b, PSS2)

        # ================= phase 2: GDFN =================
        with tc.tile_pool(name="ph2w", bufs=1) as W2:
            diag12 = {}
            with tc.tile_pool(name="ph2setup", bufs=2) as SP2, \
                 tc.tile_pool(name="pss3", bufs=2, space="PSUM") as PSS3:
                w12f = SP2.tile([C, M2], F32, tag="w12f")
                nc.sync.dma_start(w12f[:], T["w12T"])
                nc.vector.tensor_scalar(w12b[:], w12f[:], e.alpha[:], None, ALU.mult)
                for ci in range(6):
                    sz = FF_SIZES[ci]
                    cs = slice(FF_OFFS[ci], FF_OFFS[ci] + sz)
                    bp = PSS3.tile([C, 1], F32, tag="small")
                    nc.tensor.matmul(bp[:sz, :], w12f[:, cs], e.beta[:],
                                     start=True, stop=True)
                    nc.vector.tensor_copy(bias12[ci][:], bp[:sz, :])
                for ci in range(6):
                    if PH2_TAPS_PE[ci] == 0:
                        continue
                    sz = FF_SIZES[ci]
                    dl = []
                    for i in range(9):
                        d = W2.tile([sz, sz], BF16, tag=f"d12_{ci}_{i}", name=f"d12_{ci}_{i}")
                        nc.vector.tensor_scalar(d[:], e.identb[:sz, :sz],
                                                dw12t[ci][:, i:i + 1], None, ALU.mult)
                        dl.append(d)
                    diag12[ci] = dl

            with tc.tile_pool(name="ph2", bufs=2) as H2, \
                 tc.tile_pool(name="gbp", bufs=3) as GBP, \
                 tc.tile_pool(name="ph2mm", bufs=3, space="PSUM") as MMP2, \
                 tc.tile_pool(name="ph2cv", bufs=3, space="PSUM") as CVP2, \
                 tc.tile_pool(name="ph2o", bufs=2, space="PSUM") as OPP:
                for t in range(TILES):
                    mm_lo = max(t * R - 1, 0) * WW
                    mm_hi = min((t + 1) * R + 1, HH) * WW
                    c0 = t * R * WW

                    strips = []
                    for ci in range(6):
                        sz = FF_SIZES[ci]
                        s = H2.tile([sz, SR * SW], BF16, tag=f"f{ci}", name=f"f{ci}")
                        strips.append(_strip_prep(nc, s, sz, t))

                    for (a0, ncols) in _mm_blocks(mm_lo, mm_hi):
                        nrows = ncols // WW
                        sr0 = a0 // WW - t * R + 1
                        for ci in range(6):
                            sz = FF_SIZES[ci]
                            cs = slice(FF_OFFS[ci], FF_OFFS[ci] + sz)
                            ps = MMP2.tile([C, 512], F32, tag="mm2")
                            nc.tensor.matmul(ps[:sz, :ncols], w12b[:, cs],
                                             xb[:, a0:a0 + ncols],
                                             start=True, stop=True)
                            nc.scalar.activation(
                                strips[ci][:, sr0:sr0 + nrows, 1:1 + WW],
                                ps[:sz, :ncols].rearrange("p (r w) -> p r w", w=WW),
                                AF.Identity, bias=bias12[ci][:])

                    # full-tile DVE accumulators for chunks with DVE taps
                    dve_acc = {}
                    for ci in range(6):
                        ptaps = PH2_TAPS_PE[ci]
                        if ptaps == 9:
                            continue
                        sz = FF_SIZES[ci]
                        acc = H2.tile([128, R * WW], BF16, tag=f"acc{ci}",
                                      name=f"acc{ci}")
                        av = acc[:sz, :].rearrange("p (r w) -> p r w", w=WW)
                        nc.vector.tensor_scalar(
                            av, _win(strips[ci], ptaps, 0, R),
                            dw12t[ci][:, ptaps:ptaps + 1], None, ALU.mult)
                        for ti in range(ptaps + 1, 9):
                            nc.vector.scalar_tensor_tensor(
                                av, _win(strips[ci], ti, 0, R),
                                dw12t[ci][:, ti:ti + 1], av, ALU.mult, ALU.add)
                        dve_acc[ci] = acc

                    for b in range(NB):
                        r0 = 4 * b
                        bs = slice(512 * b, 512 * (b + 1))

                        def conv_val(ci, want_sbuf=False):
                            """conv value of chunk ci for this 512-block."""
                            sz = FF_SIZES[ci]
                            ptaps = PH2_TAPS_PE[ci]
                            if ptaps == 0:
                                return dve_acc[ci][:sz, bs]
                            pc = CVP2.tile([C, 512], F32, tag="cv2", name="pc")
                            for ti in range(ptaps):
                                nc.tensor.matmul(pc[:sz, :], diag12[ci][ti][:],
                                                 _win(strips[ci], ti, r0, 4),
                                                 start=(ti == 0),
                                                 stop=(ti == ptaps - 1))
                            if ptaps == 9:
                                if not want_sbuf:
                                    return pc[:sz, :]
                                ev = GBP.tile([128, 512], BF16, tag="ev", name="ev")
                                nc.scalar.copy(ev[:sz, :], pc[:sz, :])
                                return ev[:sz, :]
                            mg = GBP.tile([128, 512], BF16, tag="mg", name="mg")
                            nc.vector.tensor_tensor(mg[:sz, :], dve_acc[ci][:sz, bs],
                                                    pc[:sz, :], ALU.add)
                            return mg[:sz, :]

                        gblk = []
                        for i in range(3):
                            ci1, ci2 = i, i + 3
                            sz = FF_SIZES[i]
                            v1 = conv_val(ci1)
                            glb = GBP.tile([128, 512], BF16, tag="glb", name="glb")
                            nc.scalar.activation(glb[:sz, :], v1, AF.Gelu)
                            v2 = conv_val(ci2, want_sbuf=True)
                            gb = GBP.tile([128, 512], BF16, tag="gb", name="gb")
                            nc.vector.tensor_tensor(gb[:sz, :], glb[:sz, :],
                                                    v2, ALU.mult)
                            gblk.append(gb)

                        op = OPP.tile([C, 512], F32, tag="op")
                        for kc, ks in enumerate((128, 128, 84)):
                            nc.tensor.matmul(op[:], pwo_b[kc][:], gblk[kc][:ks, :],
                                             start=(kc == 0), stop=(kc == 2))
                        ob = GBP.tile([C, 512], F32, tag="ob")
                        nc.vector.tensor_tensor(ob[:], x[:, c0 + 512 * b:c0 + 512 * (b + 1)],
                                                op[:], ALU.add)
                        nc.sync.dma_start(T["out"][:, c0 + 512 * b:c0 + 512 * (b + 1)],
                                          ob[:])


# ======================= host-side wrapper =======================

def _prep(inputs):
    f32 = np.float32
    w = {}
    w["wqkvT"] = np.ascontiguousarray(np.asarray(inputs["pwqkv_w"], f32).T)
    w["dwqkv"] = np.ascontiguousarray(np.asarray(inputs["dwqkv_w"], f32).reshape(3 * C, 9))
    attn_s = float(np.asarray(inputs["attn_scale"]))
    ff_s = float(np.asarray(inputs["ff_scale"]))
    w["projT_s"] = np.ascontiguousarray(np.asarray(inputs["proj_w"], f32).T * attn_s)
    w["w12T"] = np.ascontiguousarray(np.asarray(inputs["pw12_w"], f32).T)
    w["dw12"] = np.ascontiguousarray(np.asarray(inputs["dw12_w"], f32).reshape(M2, 9))
    w["pwoT_s"] = np.ascontiguousarray(np.asarray(inputs["pwo_w"], f32).T * ff_s)
    w["n1w"] = np.asarray(inputs["norm1_w"], f32).reshape(C, 1).copy()
    w["n1b"] = np.asarray(inputs["norm1_b"], f32).reshape(C, 1).copy()
    w["n2w"] = np.asarray(inputs["norm2_w"], f32).reshape(C, 1).copy()
    w["n2b"] = np.asarray(inputs["norm2_b"], f32).reshape(C, 1).copy()
    w["tau"] = np.full((C, 1), float(np.asarray(inputs["temperature"])), f32)
    w["ident"] = np.eye(C, dtype=ml_dtypes.bfloat16)
    hm = np.zeros((C, C), f32)
    for h in range(HEADS):
        hm[HD * h:HD * (h + 1), HD * h:HD * (h + 1)] = 1.0
    w["hmask"] = hm
    w["onesm"] = np.ones((C, C), f32)
    return w


LAST_RESULT = {}


class _Runner:
    """Persistent sharded PJRT callable for a compiled Bacc graph (so repeated
    calls don't re-trace jit; enables accurate relative timing)."""

    def __init__(self, nc):
        import jax
        from jax.sharding import Mesh, PartitionSpec
        from jax.experimental.shard_map import shard_map
        from concourse.bass2jax import _bass_exec_p, install_neuronx_cc_hook

        install_neuronx_cc_hook()
        self.nc = nc
        in_names, out_names, out_avals, zero_outs = [], [], [], []
        partition_name = nc.partition_id_tensor.name if nc.partition_id_tensor else None
        for alloc in nc.m.functions[0].allocations:
            if not isinstance(alloc, mybir.MemoryLocationSet):
                continue
            name = alloc.memorylocations[0].name
            if alloc.kind == "ExternalInput":
                if name != partition_name:
                    in_names.append(name)
            elif alloc.kind == "ExternalOutput":
                import jax as _jax
                shape = tuple(alloc.tensor_shape)
                dtype = mybir.dt.np(alloc.dtype)
                out_avals.append(_jax.core.ShapedArray(shape, dtype))
                out_names.append(name)
                zero_outs.append(np.zeros(shape, dtype))
        self.n_params = len(in_names)
        n_outs = len(out_avals)
        self.in_param_names = list(in_names)
        in_names = in_names + out_names
        if partition_name is not None:
            in_names.append(partition_name)
        self.out_names = out_names
        self.out_avals = out_avals
        self.zero_outs = zero_outs
        donate = tuple(range(self.n_params, self.n_params + n_outs))

        def _body(*args):
            operands = list(args)
            if partition_name is not None:
                from concourse.bass2jax import partition_id_tensor
                operands.append(partition_id_tensor())
            outs = _bass_exec_p.bind(
                *operands, out_avals=tuple(out_avals), in_names=tuple(in_names),
                out_names=tuple(out_names), lowering_input_output_aliases=(),
                sim_require_finite=True, sim_require_nnan=True, nc=nc)
            return tuple(outs)

        devices = jax.devices()[:B]
        mesh = Mesh(np.asarray(devices), ("core",))
        in_specs = (PartitionSpec("core"),) * (self.n_params + n_outs)
        out_specs = (PartitionSpec("core"),) * len(out_names)
        del donate
        self.mesh = mesh
        self.pspec = PartitionSpec("core")
        self.fn = jax.jit(
            shard_map(_body, mesh=mesh, in_specs=in_specs, out_specs=out_specs,
                      check_rep=False),
            keep_unused=True)
        self._dev_args = None

    def _concat_args(self, in_maps):
        concat_in = [
            np.concatenate([np.asarray(in_maps[c][nm]) for c in range(B)], axis=0)
            for nm in self.in_param_names
        ]
        concat_zeros = [np.zeros((B * z.shape[0], *z.shape[1:]), z.dtype)
                        for z in self.zero_outs]
        return concat_in + concat_zeros

    def device_args(self, in_maps):
        """device-resident, properly sharded args (cached) for timing runs."""
        if self._dev_args is None:
            import jax
            from jax.sharding import NamedSharding
            sh = NamedSharding(self.mesh, self.pspec)
            self._dev_args = [jax.device_put(a, sh) for a in self._concat_args(in_maps)]
        return self._dev_args

    def call_raw(self, args):
        import jax
        out = self.fn(*args)
        jax.block_until_ready(out)
        return out

    def __call__(self, in_maps):
        out_arrs = self.fn(*self._concat_args(in_maps))
        return [
            {nm: np.asarray(out_arrs[i]).reshape(B, *self.out_avals[i].shape)[c]
             for i, nm in enumerate(self.out_names)}
            for c in range(B)
        ]


def _get_runner(iters=1):
    key = ("runner", iters)
    if key not in _CACHED:
        _CACHED[key] = _Runner(_build(iters))
    return _CACHED[key]


def _in_maps(inputs):
    w = _prep(inputs)
    xfull = np.asarray(inputs["x"], np.float32).reshape(B, C, N)
    in_maps = []
    for i in range(B):
        m = dict(w)
        m["x"] = np.ascontiguousarray(xfull[i])
        in_maps.append(m)
    return in_maps


def kernel(**inputs):
    in_maps = _in_maps(inputs)
    try:
        runner = _get_runner(1)
        results = runner(in_maps)
    except Exception:
        res = run_bass_kernel_spmd(_build(1), in_maps, core_ids=list(range(B)))
        results = res.results
    out = np.stack([np.asarray(results[i]["out"]).reshape(C, HH, WW)
                    for i in range(B)])
    return out


# revision 24
# speedup vs baseline: 1.1157x; 1.1157x over previous
"""Trainium2 Bass kernel for a Restormer-style block (MDTA channel attention + GDFN).

Sharding: pure data parallel, one sample per NeuronCore (B=8 samples, 8 cores).
Per-core layout: channels on partitions, flattened spatial on free dim: [128, 16384].

Per core:
  GN1 stats -> fold (alpha,beta) into W_qkv -> tiled: pw matmul -> zero-padded row
  strips -> depthwise 3x3 (fused shift-MAC taps on DVE / diagonal-matmul PSUM
  accumulation on PE) -> q,k,v. l2 row norms + Gram(q,k) via PE transposes ->
  softmax -> M^T = A^T (attn_scale*Wproj)^T -> x2 = x + M@v -> GN2 -> fold into
  W_12 -> tiled: pw -> dw3x3 -> gelu-gate -> pw_out -> out = x2 + ff.
"""

import numpy as np
import ml_dtypes

import concourse.bass as bass
import concourse.bacc as bacc
import concourse.mybir as mybir
import concourse.tile as tile
from concourse.bass_utils import run_bass_kernel_spmd

F32 = mybir.dt.float32
BF16 = mybir.dt.bfloat16
AF = mybir.ActivationFunctionType
ALU = mybir.AluOpType
AX = mybir.AxisListType

B = 8
C = 128
HEADS = 8
HD = C // HEADS          # 16
HH = 128                 # image height
WW = 128                 # image width
N = HH * WW              # 16384
MID = int(C * 2.66)      # 340
M2 = 2 * MID             # 680
EPS_GN = 1e-5
EPS_L2 = 1e-12

R = 16                   # image rows per tile
TILES = HH // R
SR = R + 2               # strip rows (halo)
SW = WW + 2              # strip width (zero-pad cols)
NB = R * WW // 512       # 512-col blocks per tile (4)

# ff chunking aligned so (t1_i, t2_i) pairs share partitions
FF_SIZES = [128, 128, 84, 128, 128, 84]
FF_OFFS = [0, 128, 256, 340, 468, 596]

PH1_OWN = {"q": "dve", "k": "pe", "v": "pe"}
PH2_OWN = ["dve", "dve", "pe", "pe", "pe", "pe"]
# leading taps per ff chunk computed on PE (rest on DVE, merged via TT add)
PH2_TAPS_PE = [0, 5, 9, 9, 9, 9]

_CACHED = {}


def _mm_blocks(lo, hi, step=512):
    out = []
    a = lo
    while a < hi:
        out.append((a, min(step, hi - a)))
        a += step
    return out


def _build(iters=1):
    key = ("nc", iters)
    if key in _CACHED:
        return _CACHED[key]
    nc = bacc.Bacc("TRN2", target_bir_lowering=False, debug=False, num_devices=B)

    T = {}
    T["x"] = nc.dram_tensor("x", [C, N], F32, kind="ExternalInput").ap()
    T["wqkvT"] = nc.dram_tensor("wqkvT", [C, 3 * C], F32, kind="ExternalInput").ap()
    T["dwqkv"] = nc.dram_tensor("dwqkv", [3 * C, 9], F32, kind="ExternalInput").ap()
    T["projT_s"] = nc.dram_tensor("projT_s", [C, C], F32, kind="ExternalInput").ap()
    T["w12T"] = nc.dram_tensor("w12T", [C, M2], F32, kind="ExternalInput").ap()
    T["dw12"] = nc.dram_tensor("dw12", [M2, 9], F32, kind="ExternalInput").ap()
    T["pwoT_s"] = nc.dram_tensor("pwoT_s", [MID, C], F32, kind="ExternalInput").ap()
    for nm in ("n1w", "n1b", "n2w", "n2b", "tau"):
        T[nm] = nc.dram_tensor(nm, [C, 1], F32, kind="ExternalInput").ap()
    T["ident"] = nc.dram_tensor("ident", [C, C], BF16, kind="ExternalInput").ap()
    T["hmask"] = nc.dram_tensor("hmask", [C, C], F32, kind="ExternalInput").ap()
    T["onesm"] = nc.dram_tensor("onesm", [C, C], F32, kind="ExternalInput").ap()
    T["out"] = nc.dram_tensor("out", [C, N], F32, kind="ExternalOutput").ap()

    with tile.TileContext(nc) as tc:
        for _ in range(iters):
            _emit(tc, T)
    nc.compile()
    _CACHED[key] = nc
    return nc


class Env:
    pass


def _gn_chunk(tc, e, x, xb, j, scr_pool):
    """GN stats + bf16 cast for 4096-col chunk j."""
    nc = tc.nc
    sl = slice(4096 * j, 4096 * (j + 1))
    nc.vector.tensor_reduce(e.s1p[:, j:j + 1], x[:, sl], AX.X, ALU.add)
    scr = scr_pool.tile([C, 4096], BF16, tag="gnscr")
    nc.scalar.activation(scr[:], x[:, sl], AF.Square, accum_out=e.s2p[:, j:j + 1])
    nc.vector.tensor_copy(xb[:, sl], x[:, sl])


def _gn_combine(tc, e, nw, nb_, ps_pool):
    nc = tc.nc
    nc.vector.tensor_reduce(e.stats2[:, 0:1], e.s1p[:], AX.X, ALU.add)
    nc.vector.tensor_reduce(e.stats2[:, 1:2], e.s2p[:], AX.X, ALU.add)
    sps = ps_pool.tile([C, 2], F32, tag="small")
    nc.tensor.matmul(sps[:], e.onesf[:], e.stats2[:], start=True, stop=True)
    nc.vector.tensor_copy(e.sums[:], sps[:])
    inv = 1.0 / (C * N)
    nc.vector.tensor_scalar_mul(e.mean[:], e.sums[:, 0:1], inv)
    nc.vector.tensor_scalar_mul(e.ex2[:], e.sums[:, 1:2], inv)
    nc.vector.tensor_mul(e.msq[:], e.mean[:], e.mean[:])
    nc.vector.tensor_sub(e.var[:], e.ex2[:], e.msq[:])
    nc.vector.tensor_scalar_add(e.var[:], e.var[:], EPS_GN)
    nc.scalar.activation(e.std[:], e.var[:], AF.Sqrt)
    nc.vector.reciprocal(e.rstd[:], e.std[:])
    nc.vector.tensor_mul(e.alpha[:], nw[:], e.rstd[:])
    nc.vector.tensor_mul(e.tmp1[:], e.mean[:], e.alpha[:])
    nc.vector.tensor_sub(e.beta[:], nb_[:], e.tmp1[:])


def _gn_fold(tc, e, x, xb, nw, nb_, scr_pool, ps_pool):
    for j in range(4):
        _gn_chunk(tc, e, x, xb, j, scr_pool)
    _gn_combine(tc, e, nw, nb_, ps_pool)


def _strip_prep(nc, strip, sz, t):
    sv = strip[:sz, :].rearrange("p (r w) -> p r w", w=SW)
    nc.vector.memset(sv[:, :, 0:1], 0.0)
    nc.vector.memset(sv[:, :, SW - 1:SW], 0.0)
    if t == 0:
        nc.vector.memset(sv[:, 0:1, :], 0.0)
    if t == TILES - 1:
        nc.vector.memset(sv[:, SR - 1:SR, :], 0.0)
    return sv


def _win(sv, i, r0, nr):
    dy, dx = i // 3, i % 3
    return sv[:, dy + r0:dy + r0 + nr, dx:dx + WW]


def _emit(tc, T):
    nc = tc.nc
    e = Env()

    with tc.tile_pool(name="persist", bufs=1) as P:
        x = P.tile([C, N], F32, tag="x")
        xb = P.tile([C, N], BF16, tag="xb")

        e.identb = P.tile([C, C], BF16, tag="identb")
        e.onesf = P.tile([C, C], F32, tag="onesf")
        projb = P.tile([C, C], BF16, tag="projb")
        wqkvb = P.tile([C, 3 * C], BF16, tag="wqkvb")
        w12b = P.tile([C, M2], BF16, tag="w12b")
        pwo_b = [P.tile([ks, C], BF16, tag=f"pwo{i}", name=f"pwo{i}") for i, ks in enumerate((128, 128, 84))]
        dwq = [P.tile([C, 9], F32, tag=f"dwq{i}", name=f"dwq{i}") for i in range(3)]
        dw12t = [P.tile([FF_SIZES[i], 9], F32, tag=f"dw12_{i}", name=f"dw12_{i}") for i in range(6)]
        biasq = [P.tile([C, 1], F32, tag=f"biasq{i}", name=f"biasq{i}") for i in range(3)]
        bias12 = [P.tile([FF_SIZES[i], 1], F32, tag=f"bias12_{i}", name=f"bias12_{i}") for i in range(6)]

        n1w = P.tile([C, 1], F32, tag="n1w")
        n1b = P.tile([C, 1], F32, tag="n1b")
        n2w = P.tile([C, 1], F32, tag="n2w")
        n2b = P.tile([C, 1], F32, tag="n2b")
        tau = P.tile([C, 1], F32, tag="tau")

        for nm in ("s1p", "s2p"):
            setattr(e, nm, P.tile([C, 4], F32, tag=nm, name=nm))
        e.stats2 = P.tile([C, 2], F32, tag="stats2")
        e.sums = P.tile([C, 2], F32, tag="sums")
        for nm in ("mean", "ex2", "msq", "var", "std", "rstd", "alpha", "beta",
                   "tmp1", "rq", "rk", "sumE", "rE"):
            setattr(e, nm, P.tile([C, 1], F32, tag=nm, name=nm))
        sqp = P.tile([C, TILES], F32, tag="sqp")
        skp = P.tile([C, TILES], F32, tag="skp")
        diagk = None
        diagv = None
        gram_s = P.tile([C, C], F32, tag="gram_s")
        gA = P.tile([C, C], BF16, tag="gA")
        gB = P.tile([C, C], BF16, tag="gB")
        Ef = P.tile([C, C], F32, tag="Ef")
        hmask = P.tile([C, C], F32, tag="hmask")
        A_full = P.tile([C, C], BF16, tag="A_full")
        mT = P.tile([C, C], BF16, tag="mT")

        # ---- constant DMAs ----
        nc.sync.dma_start(e.identb[:], T["ident"])
        nc.sync.dma_start(hmask[:], T["hmask"])
        nc.sync.dma_start(e.onesf[:], T["onesm"])
        for nm, t_ in (("n1w", n1w), ("n1b", n1b), ("n2w", n2w), ("n2b", n2b),
                       ("tau", tau)):
            nc.sync.dma_start(t_[:], T[nm])
        for i in range(3):
            nc.sync.dma_start(dwq[i][:], T["dwqkv"][128 * i:128 * (i + 1), :])
        for i in range(6):
            nc.sync.dma_start(dw12t[i][:],
                              T["dw12"][FF_OFFS[i]:FF_OFFS[i] + FF_SIZES[i], :])

        with tc.tile_pool(name="vpool", bufs=1) as VP:
            v_full = VP.tile([C, N], BF16, tag="v_full")
            diagk = [VP.tile([C, C], BF16, tag=f"diagk{i}", name=f"diagk{i}") for i in range(9)]
            diagv = [VP.tile([C, C], BF16, tag=f"diagv{i}", name=f"diagv{i}") for i in range(9)]

            # ================= setup: load x, GN1, fold =================
            with tc.tile_pool(name="setup", bufs=2) as SP, \
                 tc.tile_pool(name="pss", bufs=2, space="PSUM") as PSS:
                ko = [0, 128, 256]
                for i, ks in enumerate((128, 128, 84)):
                    pwof = SP.tile([ks, C], F32, tag="pwof")
                    nc.sync.dma_start(pwof[:], T["pwoT_s"][ko[i]:ko[i] + ks, :])
                    nc.vector.tensor_copy(pwo_b[i][:], pwof[:ks, :])
                projf = SP.tile([C, C], F32, tag="projf")
                nc.sync.dma_start(projf[:], T["projT_s"])
                nc.vector.tensor_copy(projb[:], projf[:])

                for j in range(4):
                    sl = slice(4096 * j, 4096 * (j + 1))
                    nc.sync.dma_start(x[:, sl], T["x"][:, sl])
                _gn_fold(tc, e, x, xb, n1w, n1b, SP, PSS)

                wtmp = SP.tile([C, 3 * C], F32, tag="wtmp")
                nc.sync.dma_start(wtmp[:], T["wqkvT"])
                nc.vector.tensor_scalar(wqkvb[:], wtmp[:], e.alpha[:], None, ALU.mult)
                for ci in range(3):
                    bp = PSS.tile([C, 1], F32, tag="small")
                    nc.tensor.matmul(bp[:], wtmp[:, 128 * ci:128 * (ci + 1)], e.beta[:],
                                     start=True, stop=True)
                    nc.vector.tensor_copy(biasq[ci][:], bp[:])
                for i in range(9):
                    nc.vector.tensor_scalar(diagk[i][:], e.identb[:], dwq[1][:, i:i + 1],
                                            None, ALU.mult)
                    nc.vector.tensor_scalar(diagv[i][:], e.identb[:], dwq[2][:, i:i + 1],
                                            None, ALU.mult)

            # ================= phase 1: MDTA =================
            with tc.tile_pool(name="gramp", bufs=1, space="PSUM") as GRAMP:
                gram = GRAMP.tile([C, C], F32, tag="gram")
                with tc.tile_pool(name="ph1", bufs=2) as H1, \
                     tc.tile_pool(name="ph1mm", bufs=3, space="PSUM") as MMP, \
                     tc.tile_pool(name="ph1cv", bufs=2, space="PSUM") as CVP, \
                     tc.tile_pool(name="ph1tp", bufs=2, space="PSUM") as TPP:
                    for t in range(TILES):
                        mm_lo = max(t * R - 1, 0) * WW
                        mm_hi = min((t + 1) * R + 1, HH) * WW
                        c0 = t * R * WW

                        strips = []
                        for name in "qkv":
                            s = H1.tile([C, SR * SW], BF16, tag=f"s{name}", name=f"s{name}")
                            strips.append(_strip_prep(nc, s, C, t))

                        for (a0, ncols) in _mm_blocks(mm_lo, mm_hi):
                            nrows = ncols // WW
                            sr0 = a0 // WW - t * R + 1
                            for ci in range(3):
                                ps = MMP.tile([C, 512], F32, tag="mm")
                                nc.tensor.matmul(
                                    ps[:, :ncols],
                                    wqkvb[:, 128 * ci:128 * (ci + 1)],
                                    xb[:, a0:a0 + ncols], start=True, stop=True)
                                nc.scalar.activation(
                                    strips[ci][:, sr0:sr0 + nrows, 1:1 + WW],
                                    ps[:, :ncols].rearrange("p (r w) -> p r w", w=WW),
                                    AF.Identity, bias=biasq[ci][:])

                        # q: DVE fused taps
                        q_acc = H1.tile([C, R * WW], BF16, tag="q_acc")
                        qv = q_acc[:].rearrange("p (r w) -> p r w", w=WW)
                        nc.vector.tensor_scalar(qv, _win(strips[0], 0, 0, R),
                                                dwq[0][:, 0:1], None, ALU.mult)
                        for i in range(1, 9):
                            nc.vector.scalar_tensor_tensor(
                                qv, _win(strips[0], i, 0, R), dwq[0][:, i:i + 1],
                                qv, ALU.mult, ALU.add)

                        # k, v: PE diagonal-matmul taps
                        k_acc = H1.tile([C, R * WW], BF16, tag="k_acc")
                        for b in range(NB):
                            r0 = 4 * b
                            pk = CVP.tile([C, 512], F32, tag="cv")
                            for i in range(9):
                                nc.tensor.matmul(pk[:], diagk[i][:],
                                                 _win(strips[1], i, r0, 4),
                                                 start=(i == 0), stop=(i == 8))
                            nc.scalar.copy(k_acc[:, 512 * b:512 * (b + 1)], pk[:])
                            pv = CVP.tile([C, 512], F32, tag="cv")
                            for i in range(9):
                                nc.tensor.matmul(pv[:], diagv[i][:],
                                                 _win(strips[2], i, r0, 4),
                                                 start=(i == 0), stop=(i == 8))
                            nc.scalar.copy(v_full[:, c0 + 512 * b:c0 + 512 * (b + 1)],
                                           pv[:])

                        # transposes + Gram; qT/kT double as Square scratch
                        qT = H1.tile([C, R * WW], BF16, tag="qT")
                        kT = H1.tile([C, R * WW], BF16, tag="kT")
                        nc.scalar.activation(qT[:], q_acc[:], AF.Square,
                                             accum_out=sqp[:, t:t + 1])
                        nc.scalar.activation(kT[:], k_acc[:], AF.Square,
                                             accum_out=skp[:, t:t + 1])
                        ntp = R * WW // C
                        for j4 in range(ntp // 4):
                            tq = TPP.tile([C, 512], BF16, tag="tp")
                            tk = TPP.tile([C, 512], BF16, tag="tp")
                            for jj in range(4):
                                j = 4 * j4 + jj
                                nc.tensor.transpose(tq[:, 128 * jj:128 * (jj + 1)],
                                                    q_acc[:, 128 * j:128 * (j + 1)],
                                                    e.identb[:])
                                nc.tensor.transpose(tk[:, 128 * jj:128 * (jj + 1)],
                                                    k_acc[:, 128 * j:128 * (j + 1)],
                                                    e.identb[:])
                            nc.scalar.copy(qT[:, 512 * j4:512 * (j4 + 1)], tq[:])
                            nc.scalar.copy(kT[:, 512 * j4:512 * (j4 + 1)], tk[:])
                        for j in range(ntp):
                            nc.tensor.matmul(
                                gram[:], qT[:, 128 * j:128 * (j + 1)],
                                kT[:, 128 * j:128 * (j + 1)],
                                start=(t == 0 and j == 0),
                                stop=(t == TILES - 1 and j == ntp - 1))

                # l2 norms -> rq, rk (rk includes temperature)
                for parts, dst in ((sqp, e.rq), (skp, e.rk)):
                    nc.vector.tensor_reduce(dst[:], parts[:], AX.X, ALU.add)
                    nc.scalar.activation(dst[:], dst[:], AF.Sqrt)
                    nc.vector.tensor_scalar_max(dst[:], dst[:], EPS_L2)
                    nc.vector.reciprocal(dst[:], dst[:])
                nc.vector.tensor_mul(e.rk[:], e.rk[:], tau[:])
                nc.vector.tensor_copy(gram_s[:], gram[:])

            # ============ attention finalize + x2 + GN2 ============
            with tc.tile_pool(name="attp", bufs=2, space="PSUM") as ATP, \
                 tc.tile_pool(name="midp", bufs=2) as MP:
                nc.vector.tensor_scalar(gA[:], gram_s[:], e.rq[:], None, ALU.mult)
                tp1 = ATP.tile([C, C], BF16, tag="tpa")
                nc.tensor.transpose(tp1[:], gA[:], e.identb[:])
                nc.vector.tensor_scalar(gB[:], tp1[:], e.rk[:], None, ALU.mult)
                tp2 = ATP.tile([C, C], BF16, tag="tpa")
                nc.tensor.transpose(tp2[:], gB[:], e.identb[:])
                nc.scalar.activation(Ef[:], tp2[:], AF.Exp)
                nc.vector.tensor_tensor(Ef[:], Ef[:], hmask[:], ALU.mult)
                nc.vector.tensor_reduce(e.sumE[:], Ef[:], AX.X, ALU.add)
                nc.vector.reciprocal(e.rE[:], e.sumE[:])
                nc.vector.tensor_scalar(A_full[:], Ef[:], e.rE[:], None, ALU.mult)
                mtp = ATP.tile([C, C], F32, tag="tpa")
                nc.tensor.matmul(mtp[:], A_full[:], projb[:], start=True, stop=True)
                nc.scalar.copy(mT[:], mtp[:])

                with tc.tile_pool(name="pss2", bufs=2, space="PSUM") as PSS2:
                    for a0 in range(0, N, 512):
                        ap_ = ATP.tile([C, 512], F32, tag="apv", bufs=4)
                        nc.tensor.matmul(ap_[:], mT[:], v_full[:, a0:a0 + 512],
                                         start=True, stop=True)
                        nc.vector.tensor_tensor(x[:, a0:a0 + 512], x[:, a0:a0 + 512],
                                                ap_[:], ALU.add)
                        if a0 % 4096 == 4096 - 512:
                            _gn_chunk(tc, e, x, xb, a0 // 4096, MP)
                    _gn_combine(tc, e, n2w, n2b, PSS2)

        # ================= phase 2: GDFN =================
        with tc.tile_pool(name="ph2w", bufs=1) as W2:
            diag12 = {}
            with tc.tile_pool(name="ph2setup", bufs=2) as SP2, \
                 tc.tile_pool(name="pss3", bufs=2, space="PSUM") as PSS3:
                w12f = SP2.tile([C, M2], F32, tag="w12f")
                nc.sync.dma_start(w12f[:], T["w12T"])
                nc.vector.tensor_scalar(w12b[:], w12f[:], e.alpha[:], None, ALU.mult)
                for ci in range(6):
                    sz = FF_SIZES[ci]
                    cs = slice(FF_OFFS[ci], FF_OFFS[ci] + sz)
                    bp = PSS3.tile([C, 1], F32, tag="small")
                    nc.tensor.matmul(bp[:sz, :], w12f[:, cs], e.beta[:],
                                     start=True, stop=True)
                    nc.vector.tensor_copy(bias12[ci][:], bp[:sz, :])
                for ci in range(6):
                    if PH2_TAPS_PE[ci] == 0:
                        continue
                    sz = FF_SIZES[ci]
                    dl = []
                    for i in range(9):
                        d = W2.tile([sz, sz], BF16, tag=f"d12_{ci}_{i}", name=f"d12_{ci}_{i}")
                        nc.vector.tensor_scalar(d[:], e.identb[:sz, :sz],
                                                dw12t[ci][:, i:i + 1], None, ALU.mult)
                        dl.append(d)
                    diag12[ci] = dl

            with tc.tile_pool(name="ph2", bufs=2) as H2, \
                 tc.tile_pool(name="gbp", bufs=2) as GBP, \
                 tc.tile_pool(name="ph2mm", bufs=3, space="PSUM") as MMP2, \
                 tc.tile_pool(name="ph2cv", bufs=3, space="PSUM") as CVP2, \
                 tc.tile_pool(name="ph2o", bufs=2, space="PSUM") as OPP:
                for t in range(TILES):
                    mm_lo = max(t * R - 1, 0) * WW
                    mm_hi = min((t + 1) * R + 1, HH) * WW
                    c0 = t * R * WW

                    strips = []
                    for ci in range(6):
                        sz = FF_SIZES[ci]
                        s = H2.tile([sz, SR * SW], BF16, tag=f"f{ci}", name=f"f{ci}", bufs=(3 if ci >= 3 else 2))
                        strips.append(_strip_prep(nc, s, sz, t))

                    for (a0, ncols) in _mm_blocks(mm_lo, mm_hi):
                        nrows = ncols // WW
                        sr0 = a0 // WW - t * R + 1
                        for ci in range(6):
                            sz = FF_SIZES[ci]
                            cs = slice(FF_OFFS[ci], FF_OFFS[ci] + sz)
                            ps = MMP2.tile([C, 512], F32, tag="mm2")
                            nc.tensor.matmul(ps[:sz, :ncols], w12b[:, cs],
                                             xb[:, a0:a0 + ncols],
                                             start=True, stop=True)
                            nc.scalar.activation(
                                strips[ci][:, sr0:sr0 + nrows, 1:1 + WW],
                                ps[:sz, :ncols].rearrange("p (r w) -> p r w", w=WW),
                                AF.Identity, bias=bias12[ci][:])

                    # full-tile DVE accumulators for chunks with DVE taps
                    dve_acc = {}
                    for ci in range(6):
                        ptaps = PH2_TAPS_PE[ci]
                        if ptaps == 9:
                            continue
                        sz = FF_SIZES[ci]
                        acc = H2.tile([128, R * WW], BF16, tag=f"acc{ci}",
                                      name=f"acc{ci}", bufs=1)
                        av = acc[:sz, :].rearrange("p (r w) -> p r w", w=WW)
                        nc.vector.tensor_scalar(
                            av, _win(strips[ci], ptaps, 0, R),
                            dw12t[ci][:, ptaps:ptaps + 1], None, ALU.mult)
                        for ti in range(ptaps + 1, 9):
                            nc.vector.scalar_tensor_tensor(
                                av, _win(strips[ci], ti, 0, R),
                                dw12t[ci][:, ti:ti + 1], av, ALU.mult, ALU.add)
                        dve_acc[ci] = acc

                    for b in range(NB):
                        r0 = 4 * b
                        bs = slice(512 * b, 512 * (b + 1))

                        def conv_val(ci, want_sbuf=False):
                            """conv value of chunk ci for this 512-block."""
                            sz = FF_SIZES[ci]
                            ptaps = PH2_TAPS_PE[ci]
                            if ptaps == 0:
                                return dve_acc[ci][:sz, bs]
                            pc = CVP2.tile([C, 512], F32, tag="cv2", name="pc")
                            for ti in range(ptaps):
                                nc.tensor.matmul(pc[:sz, :], diag12[ci][ti][:],
                                                 _win(strips[ci], ti, r0, 4),
                                                 start=(ti == 0),
                                                 stop=(ti == ptaps - 1))
                            if ptaps == 9:
                                if not want_sbuf:
                                    return pc[:sz, :]
                                ev = GBP.tile([128, 512], BF16, tag="ev", name="ev")
                                nc.scalar.copy(ev[:sz, :], pc[:sz, :])
                                return ev[:sz, :]
                            mg = GBP.tile([128, 512], BF16, tag="mg", name="mg")
                            nc.vector.tensor_tensor(mg[:sz, :], dve_acc[ci][:sz, bs],
                                                    pc[:sz, :], ALU.add)
                            return mg[:sz, :]

                        gblk = []
                        for i in range(3):
                            ci1, ci2 = i, i + 3
                            sz = FF_SIZES[i]
                            v1 = conv_val(ci1)
                            glb = GBP.tile([128, 512], BF16, tag="glb", name="glb")
                            nc.scalar.activation(glb[:sz, :], v1, AF.Gelu)
                            v2 = conv_val(ci2, want_sbuf=True)
                            gb = GBP.tile([128, 512], BF16, tag="gb", name="gb")
                            nc.vector.tensor_tensor(gb[:sz, :], glb[:sz, :],
                                                    v2, ALU.mult)
                            gblk.append(gb)

                        op = OPP.tile([C, 512], F32, tag="op")
                        for kc, ks in enumerate((128, 128, 84)):
                            nc.tensor.matmul(op[:], pwo_b[kc][:], gblk[kc][:ks, :],
                                             start=(kc == 0), stop=(kc == 2))
                        ob = GBP.tile([C, 512], F32, tag="ob")
                        nc.vector.tensor_tensor(ob[:], x[:, c0 + 512 * b:c0 + 512 * (b + 1)],
                                                op[:], ALU.add)
                        nc.sync.dma_start(T["out"][:, c0 + 512 * b:c0 + 512 * (b + 1)],
                                          ob[:])


# ======================= host-side wrapper =======================

def _prep(inputs):
    f32 = np.float32
    w = {}
    w["wqkvT"] = np.ascontiguousarray(np.asarray(inputs["pwqkv_w"], f32).T)
    w["dwqkv"] = np.ascontiguousarray(np.asarray(inputs["dwqkv_w"], f32).reshape(3 * C, 9))
    attn_s = float(np.asarray(inputs["attn_scale"]))
    ff_s = float(np.asarray(inputs["ff_scale"]))
    w["projT_s"] = np.ascontiguousarray(np.asarray(inputs["proj_w"], f32).T * attn_s)
    w["w12T"] = np.ascontiguousarray(np.asarray(inputs["pw12_w"], f32).T)
    w["dw12"] = np.ascontiguousarray(np.asarray(inputs["dw12_w"], f32).reshape(M2, 9))
    w["pwoT_s"] = np.ascontiguousarray(np.asarray(inputs["pwo_w"], f32).T * ff_s)
    w["n1w"] = np.asarray(inputs["norm1_w"], f32).reshape(C, 1).copy()
    w["n1b"] = np.asarray(inputs["norm1_b"], f32).reshape(C, 1).copy()
    w["n2w"] = np.asarray(inputs["norm2_w"], f32).reshape(C, 1).copy()
    w["n2b"] = np.asarray(inputs["norm2_b"], f32).reshape(C, 1).copy()
    w["tau"] = np.full((C, 1), float(np.asarray(inputs["temperature"])), f32)
    w["ident"] = np.eye(C, dtype=ml_dtypes.bfloat16)
    hm = np.zeros((C, C), f32)
    for h in range(HEADS):
        hm[HD * h:HD * (h + 1), HD * h:HD * (h + 1)] = 1.0
    w["hmask"] = hm
    w["onesm"] = np.ones((C, C), f32)
    return w


LAST_RESULT = {}


class _Runner:
    """Persistent sharded PJRT callable for a compiled Bacc graph (so repeated
    calls don't re-trace jit; enables accurate relative timing)."""

    def __init__(self, nc):
        import jax
        from jax.sharding import Mesh, PartitionSpec
        from jax.experimental.shard_map import shard_map
        from concourse.bass2jax import _bass_exec_p, install_neuronx_cc_hook

        install_neuronx_cc_hook()
        self.nc = nc
        in_names, out_names, out_avals, zero_outs = [], [], [], []
        partition_name = nc.partition_id_tensor.name if nc.partition_id_tensor else None
        for alloc in nc.m.functions[0].allocations:
            if not isinstance(alloc, mybir.MemoryLocationSet):
                continue
            name = alloc.memorylocations[0].name
            if alloc.kind == "ExternalInput":
                if name != partition_name:
                    in_names.append(name)
            elif alloc.kind == "ExternalOutput":
                import jax as _jax
                shape = tuple(alloc.tensor_shape)
                dtype = mybir.dt.np(alloc.dtype)
                out_avals.append(_jax.core.ShapedArray(shape, dtype))
                out_names.append(name)
                zero_outs.append(np.zeros(shape, dtype))
        self.n_params = len(in_names)
        n_outs = len(out_avals)
        self.in_param_names = list(in_names)
        in_names = in_names + out_names
        if partition_name is not None:
            in_names.append(partition_name)
        self.out_names = out_names
        self.out_avals = out_avals
        self.zero_outs = zero_outs
        donate = tuple(range(self.n_params, self.n_params + n_outs))

        def _body(*args):
            operands = list(args)
            if partition_name is not None:
                from concourse.bass2jax import partition_id_tensor
                operands.append(partition_id_tensor())
            outs = _bass_exec_p.bind(
                *operands, out_avals=tuple(out_avals), in_names=tuple(in_names),
                out_names=tuple(out_names), lowering_input_output_aliases=(),
                sim_require_finite=True, sim_require_nnan=True, nc=nc)
            return tuple(outs)

        devices = jax.devices()[:B]
        mesh = Mesh(np.asarray(devices), ("core",))
        in_specs = (PartitionSpec("core"),) * (self.n_params + n_outs)
        out_specs = (PartitionSpec("core"),) * len(out_names)
        del donate
        self.mesh = mesh
        self.pspec = PartitionSpec("core")
        self.fn = jax.jit(
            shard_map(_body, mesh=mesh, in_specs=in_specs, out_specs=out_specs,
                      check_rep=False),
            keep_unused=True)
        self._dev_args = None

    def _concat_args(self, in_maps):
        concat_in = [
            np.concatenate([np.asarray(in_maps[c][nm]) for c in range(B)], axis=0)
            for nm in self.in_param_names
        ]
        concat_zeros = [np.zeros((B * z.shape[0], *z.shape[1:]), z.dtype)
                        for z in self.zero_outs]
        return concat_in + concat_zeros

    def device_args(self, in_maps):
        """device-resident, properly sharded args (cached) for timing runs."""
        if self._dev_args is None:
            import jax
            from jax.sharding import NamedSharding
            sh = NamedSharding(self.mesh, self.pspec)
            self._dev_args = [jax.device_put(a, sh) for a in self._concat_args(in_maps)]
        return self._dev_args

    def call_raw(self, args):
        import jax
        out = self.fn(*args)
        jax.block_until_ready(out)
        return out

    def __call__(self, in_maps):
        out_arrs = self.fn(*self._concat_args(in_maps))
        return [
            {nm: np.asarray(out_arrs[i]).reshape(B, *self.out_avals[i].shape)[c]
             for i, nm in enumerate(self.out_names)}
            for c in range(B)
        ]


def _get_runner(iters=1):
    key = ("runner", iters)
    if key not in _CACHED:
        _CACHED[key] = _Runner(_build(iters))
    return _CACHED[key]


def _in_maps(inputs):
    w = _prep(inputs)
    xfull = np.asarray(inputs["x"], np.float32).reshape(B, C, N)
    in_maps = []
    for i in range(B):
        m = dict(w)
        m["x"] = np.ascontiguousarray(xfull[i])
        in_maps.append(m)
    return in_maps


def kernel(**inputs):
    in_maps = _in_maps(inputs)
    try:
        runner = _get_runner(1)
        results = runner(in_maps)
    except Exception:
        res = run_bass_kernel_spmd(_build(1), in_maps, core_ids=list(range(B)))
        results = res.results
    out = np.stack([np.asarray(results[i]["out"]).reshape(C, HH, WW)
                    for i in range(B)])
    return out
